# revision 16
# baseline (speedup 1.0000x reference)
"""Trainium2 Bass kernel for EnhancedAttention (sparse band+global attention).

Problem (hardcoded): B=2, S=2048, D=1024, H=16, HD=64, band |i-j|<=32,
16 global tokens. reference returns (out [B,S,D], attn [B,H,S,S]).

Sharding: 8 cores = 2 batches x 4 head-quads. Core c handles batch c//4,
heads [4*(c%4), 4*(c%4)+4). Per core:
  - project q/k transposed ([256, 2048]) and v natural ([2048, 256]), f32r
  - per (pair) sparse attention:
      row side (attn output): scores + additive -1e9 mask via identity
      matmul into PSUM, exp with fused row-sum (accum_out), reciprocal,
      gpsimd normalize, strided DMA of just the nonzero band/global region
      (output buffers are pre-zeroed by the runtime);
      T side (for ctx^T): scores recomputed transposed in shared 128-col
      chunks, exp, then AV matmuls accumulate ctxT [64, 2048] in PSUM;
      normalization via row-recips transposed + DRAM-broadcast.
  - out-projection partial outT [1024, 2048] = (Wo cols-slice)^T @ ctxT
Host: attn shards concatenated; out = sum of 4 partials per batch + bo.
Masked attn entries are exact zeros (exp underflow), matching reference.
"""
import sys

sys.path.insert(0, "/opt/trn_rl_repo")

import numpy as np

import concourse.bass as bass
import concourse.tile as tile
from concourse import bacc, mybir
from concourse.bass_utils import run_bass_kernel_spmd

B, S, D, H = 2, 2048, 1024, 16
HD = 64          # head dim
HQ = 4           # heads per core
HC = HQ * HD     # 256 head-cols per core
G = 16           # global tokens
P = 128
NT = S // P      # 16 row tiles
KC = D // P      # 8 contraction chunks
F32 = mybir.dt.float32
F32R = mybir.dt.float32r
NEG = np.float32(-1e9)
SCALE = 0.125    # 1/sqrt(64)
T0W = 176        # tile0 band-rows score width (covers band+global of rows 16..127)


def _chunk_rows(m):
    """T-side chunk m: key-cols [128m, 128m+128), query-rows [r0, r1)."""
    return max(0, P * (m - 1)), min(S, P * (m + 2))


def _build_masks():
    """Additive 0/-1e9 masks (f32)."""
    r = np.arange(P)

    def madd(v):
        return np.where(v, np.float32(0.0), NEG)

    # tile0 band-rows piece [128, 176]: i=r, j=c
    c = np.arange(T0W)
    mask0b = madd((np.abs(r[:, None] - c[None, :]) <= 32)
                  | (r[:, None] < G) | (c[None, :] < G))
    # mid tiles 1..14: [16 global][256 window at 128t-64]; j-i = c-r-32-32
    c = np.arange(256)
    band_mid = madd((c[None, :] - r[:, None] >= 32) & (c[None, :] - r[:, None] <= 96))
    maskmid = np.concatenate([np.zeros((P, G), np.float32), band_mid], axis=1)
    # tile 15: [16 global][192 window at 1856]; j=1856+c, i=1920+r
    c = np.arange(192)
    band15 = madd((c[None, :] - r[:, None] >= 32) & (c[None, :] - r[:, None] <= 96))
    mask15 = np.concatenate([np.zeros((P, G), np.float32), band15], axis=1)

    def maskT(m):
        r0, r1 = _chunk_rows(m)
        j = P * m + np.arange(P)
        i = np.arange(r0, r1)
        v = (np.abs(i[None, :] - j[:, None]) <= 32) | (i[None, :] < G)
        v &= j[:, None] >= G
        return madd(v)

    maskT0, maskT1, maskTi = maskT(0), maskT(1), maskT(2)
    for m in range(3, 15):
        assert np.array_equal(maskT(m), maskTi)
    assert np.array_equal(maskT(15), maskTi[:, :256])
    return mask0b, maskmid, mask15, maskT0, maskT1, maskTi


def _build_module(with_bias):
    nc = bacc.Bacc("TRN2", target_bir_lowering=False, debug=False)

    def din(name, shape, dt=F32R):
        return nc.dram_tensor(name, shape, dt, kind="ExternalInput")

    tens = dict(
        xq_d=din("xqT", [D, S]), xk_d=din("xkT", [D, S]), xv_d=din("xvT", [D, S]),
        wq_d=din("wq", [D, HC]), wk_d=din("wk", [D, HC]), wv_d=din("wv", [D, HC]),
        wo_d=din("wo", [HC, D]),
        id_r_d=din("ident_r", [P, P]), id32_d=din("ident32", [P, P], F32),
        m0b_d=din("mask0b", [P, T0W]),
        mm_d=din("maskmid", [P, G + 256]), m15_d=din("mask15", [P, G + 192]),
        mt0_d=din("maskT0", [P, 256]), mt1_d=din("maskT1", [P, 384]),
        mti_d=din("maskTi", [P, 384]),
        attn_d=nc.dram_tensor("attn", [HQ, S, S], F32, kind="ExternalOutput"),
        outT_d=nc.dram_tensor("outT", [D, S], F32, kind="ExternalOutput"),
        pex_d=nc.dram_tensor("pex", [HQ, G, S], F32, kind="ExternalOutput"),
        recq_d=nc.dram_tensor("recq", [HQ, G, P], F32, kind="Internal"),
    )
    import os
    if os.environ.get("KBDEBUG"):
        tens.update(
            dbg_qT=nc.dram_tensor("dbg_qT", [P, 2, S], F32R, kind="ExternalOutput"),
            dbg_v=nc.dram_tensor("dbg_v", [P, NT, HC], F32R, kind="ExternalOutput"),
            dbg_ctxT=nc.dram_tensor("dbg_ctxT", [P, 2, S], F32R, kind="ExternalOutput"),
            dbg_rt=nc.dram_tensor("dbg_rt", [HQ, G, P], F32, kind="ExternalOutput"),
            dbg_rbc=nc.dram_tensor("dbg_rbc", [HQ, HD, S], F32, kind="ExternalOutput"),
            dbg_rp=nc.dram_tensor("dbg_rp", [HQ, P, G], F32, kind="ExternalOutput"),
        )
    if with_bias:
        tens.update(bq_d=din("bq2", [P, 2], F32), bk_d=din("bk2", [P, 2], F32),
                    bv_d=din("bvb", [P, HC], F32))

    with tile.TileContext(nc) as tc:
        _emit(nc, tc, tens, with_bias)
    nc.compile()
    return nc


def _emit(nc, tc, T, with_bias):
    from contextlib import ExitStack
    Exp = mybir.ActivationFunctionType.Exp

    ctx = ExitStack()
    with ctx:
        consts = ctx.enter_context(tc.tile_pool(name="consts", bufs=1))
        qkv = ctx.enter_context(tc.tile_pool(name="qkv", bufs=1))

        # ---- constants ----
        wq_sb = consts.tile([P, KC, HC], F32R)
        wk_sb = consts.tile([P, KC, HC], F32R)
        wv_sb = consts.tile([P, KC, HC], F32R)
        wo_sb = consts.tile([P, 2, D], F32R)
        nc.sync.dma_start(out=wq_sb, in_=T["wq_d"].ap().rearrange(
            "(kc p) n -> p kc n", p=P))
        nc.sync.dma_start(out=wk_sb, in_=T["wk_d"].ap().rearrange(
            "(kc p) n -> p kc n", p=P))
        nc.sync.dma_start(out=wv_sb, in_=T["wv_d"].ap().rearrange(
            "(kc p) n -> p kc n", p=P))
        nc.sync.dma_start(out=wo_sb, in_=T["wo_d"].ap().rearrange(
            "(cc p) n -> p cc n", p=P))

        def load_const(d):
            nm = d.ap().tensor.name
            t = consts.tile(list(d.ap().shape), d.dtype,
                            name=f"c_{nm}", tag=f"c_{nm}")
            nc.sync.dma_start(out=t, in_=d.ap())
            return t

        id_r = load_const(T["id_r_d"])
        id32 = load_const(T["id32_d"])
        mask0b = load_const(T["m0b_d"])
        maskmid = load_const(T["mm_d"])
        mask15 = load_const(T["m15_d"])
        maskT0 = load_const(T["mt0_d"])
        maskT1 = load_const(T["mt1_d"])
        maskTi = load_const(T["mti_d"])
        if with_bias:
            bq_sb = load_const(T["bq_d"])
            bk_sb = load_const(T["bk_d"])
            bv_sb = load_const(T["bv_d"])

        # ---- persistent tensors ----
        qT = qkv.tile([P, 2, S], F32R)
        kT = qkv.tile([P, 2, S], F32R)
        v_sb = qkv.tile([P, NT, HC], F32R)
        ctxT = qkv.tile([P, 2, S], F32R)

        # ================= projections =================
        with tc.tile_pool(name="xpool", bufs=2) as xpool:
            with tc.tile_pool(name="ppsum", bufs=8, space="PSUM") as ppsum:
                for which, x_d, w_sb3, out_t in (
                        ("q", T["xq_d"], wq_sb, qT), ("k", T["xk_d"], wk_sb, kT)):
                    x_view = x_d.ap().rearrange("(kc p) r -> kc p r", p=P)
                    psums = [ppsum.tile([P, 512], F32, tag="pp", name=f"pp_{which}_{i}") for i in range(8)]
                    for kc in range(KC):
                        x_sb = xpool.tile([P, S], F32R, tag="x")
                        nc.sync.dma_start(out=x_sb, in_=x_view[kc])
                        for m in range(2):
                            for n in range(4):
                                nc.tensor.matmul(
                                    psums[m * 4 + n],
                                    w_sb3[:, kc, m * P:(m + 1) * P],
                                    x_sb[:, n * 512:(n + 1) * 512],
                                    start=(kc == 0), stop=(kc == KC - 1))
                    for m in range(2):
                        for n in range(4):
                            dst = out_t[:, m, n * 512:(n + 1) * 512]
                            src = psums[m * 4 + n]
                            if with_bias:
                                bsb = bq_sb if which == "q" else bk_sb
                                nc.vector.tensor_scalar_add(dst, src, bsb[:, m:m + 1])
                            elif n % 2 == 0:
                                nc.scalar.copy(dst, src)
                            else:
                                nc.vector.tensor_copy(dst, src)
            with tc.tile_pool(name="vpsum", bufs=8, space="PSUM") as vpsum:
                x_view = T["xv_d"].ap().rearrange("(kc p) r -> kc p r", p=P)
                zz = xpool.tile([P, 512], F32, tag="zz")
                nc.vector.memset(zz, 0.0)
                vpsums = [vpsum.tile([P, 2, HC], F32, tag="vp", name=f"vp_{i}") for i in range(8)]
                for j in range(8):
                    nc.tensor.matmul(vpsums[j].rearrange("p a b -> p (a b)"),
                                     id32, zz, start=True, stop=False,
                                     skip_group_check=True)
                for kc in range(KC):
                    x_sb = xpool.tile([P, S], F32R, tag="x")
                    nc.sync.dma_start(out=x_sb, in_=x_view[kc])
                    for rt in range(NT):
                        nc.tensor.matmul(
                            vpsums[rt // 2][:, rt % 2, :],
                            x_sb[:, rt * P:(rt + 1) * P],
                            wv_sb[:, kc, :],
                            start=False,
                            stop=(kc == KC - 1 and rt % 2 == 1),
                            skip_group_check=True)
                for rt in range(NT):
                    dst = v_sb[:, rt, :]
                    src_ap = vpsums[rt // 2][:, rt % 2, :]
                    if with_bias:
                        nc.vector.tensor_add(dst, src_ap, bv_sb)
                    elif rt % 2 == 0:
                        nc.scalar.copy(dst, src_ap)
                    else:
                        nc.vector.tensor_copy(dst, src_ap)

        # ================= attention =================
        tc.no_sync_barrier()
        attn_ap = T["attn_d"].ap()
        recq_ap = T["recq_d"].ap()

        with tc.tile_pool(name="stp", bufs=1, space="PSUM") as stp, \
             tc.tile_pool(name="smallp", bufs=1, space="PSUM") as smallp, \
             tc.tile_pool(name="avp", bufs=1, space="PSUM") as avp, \
             tc.tile_pool(name="rowp", bufs=1, space="PSUM") as rowp, \
             tc.tile_pool(name="tp", bufs=3) as tp, \
             tc.tile_pool(name="tg", bufs=1) as tg, \
             tc.tile_pool(name="rowsb", bufs=3) as rowsb, \
             tc.tile_pool(name="rowbig", bufs=1) as rowbig:
            for h in range(HQ):
                p0 = (h % 2) * HD
                mh = h // 2
                hc = h * HD

                def qTh(a, b):
                    return qT[p0:p0 + HD, mh, a:b]

                def kTh(a, b):
                    return kT[p0:p0 + HD, mh, a:b]

                # ---- T-side global block ----
                pTg = tg.tile([G, 4, 512], F32R, tag="pTg")
                for n in range(4):
                    sg = smallp.tile([G, 512], F32, tag="small")
                    nc.tensor.matmul(sg, kTh(0, G), qTh(n * 512, (n + 1) * 512),
                                     start=True, stop=True)
                    nc.scalar.activation(pTg[:, n, :], sg, Exp,
                                         bias=0.0, scale=SCALE)

                # ---- AV init: ctxT_acc [64, 2048] <- v_glob^T @ PTg ----
                acc = avp.tile([HD, S], F32, tag="acc")
                for n in range(4):
                    nc.tensor.matmul(acc[:, n * 512:(n + 1) * 512],
                                     v_sb[0:G, 0, hc:hc + HD], pTg[:, n, :],
                                     start=True, stop=False)

                # ---- T-side chunks (16, mask-first) + AV accumulate ----
                for m in range(NT):
                    r0, r1 = _chunk_rows(m)
                    w = r1 - r0
                    sT = stp.tile([P, 384], F32, tag="sT", name=f"sT_{h}_{m}")
                    pT = tp.tile([P, 384], F32R, tag="pT", name=f"pT_{h}_{m}")
                    mt = (maskT0 if m == 0 else maskT1 if m == 1
                          else maskTi[:, 0:w])
                    nc.tensor.matmul(sT[:, 0:w], id_r, mt,
                                     start=True, stop=False)
                    nc.tensor.matmul(sT[:, 0:w], kTh(m * P, (m + 1) * P),
                                     qTh(r0, r1), start=False, stop=True,
                                     skip_group_check=True)
                    nc.scalar.activation(pT[:, 0:w], sT[:, 0:w], Exp,
                                         bias=0.0, scale=SCALE)
                    nc.tensor.matmul(acc[:, r0:r1], v_sb[:, m, hc:hc + HD],
                                     pT[:, 0:w], start=False, stop=False,
                                     skip_group_check=True)

                # ---- T-side strip (rows 0..16 x cols 256..2048) ----
                ss = smallp.tile([P, 14 * G], F32, tag="small")
                for m in range(2, NT):
                    nc.tensor.matmul(ss[:, (m - 2) * G:(m - 1) * G],
                                     kTh(m * P, (m + 1) * P), qTh(0, G),
                                     start=True, stop=True)
                pTs = tg.tile([P, 14 * G], F32R, tag="pTs")
                nc.scalar.activation(pTs, ss, Exp, bias=0.0, scale=SCALE)
                for m in range(2, NT):
                    nc.tensor.matmul(acc[:, 0:G], v_sb[:, m, hc:hc + HD],
                                     pTs[:, (m - 2) * G:(m - 1) * G],
                                     start=False, stop=(m == NT - 1),
                                     skip_group_check=True)

                # ---- row side ----
                rec_pack = rowsb.tile([P, NT], F32, tag="rpack")
                # tile0 dense rows 0..15: [16, 2048]
                p0d = rowbig.tile([G, 4, 512], F32, tag="p0d")
                den4 = rowsb.tile([G, 4], F32, tag="den4")
                for n in range(4):
                    s0 = smallp.tile([G, 512], F32, tag="small")
                    nc.tensor.matmul(s0, qTh(0, G), kTh(n * 512, (n + 1) * 512),
                                     start=True, stop=True)
                    nc.scalar.activation(p0d[:, n, :], s0, Exp, bias=0.0,
                                         scale=SCALE, accum_out=den4[:, n:n + 1])
                den0 = rowsb.tile([G, 1], F32, tag="dend")
                nc.vector.reduce_sum(den0, den4, axis=mybir.AxisListType.X)
                nc.sync.dma_start(out=T["pex_d"].ap()[h],
                                  in_=p0d.rearrange("g a b -> g (a b)"))
                # tile0 band rows 16..127: [128, 176]
                sb0 = rowp.tile([P, 512], F32, tag="row")
                nc.tensor.matmul(sb0[:, 0:T0W], id_r, mask0b,
                                 start=True, stop=False)
                nc.tensor.matmul(sb0[:, 0:T0W], qTh(0, P), kTh(0, T0W),
                                 start=False, stop=True,
                                 skip_group_check=True)
                pb0 = rowsb.tile([P, G + 256], F32, tag="pb")
                denb = rowsb.tile([P, 1], F32, tag="den")
                nc.scalar.activation(pb0[:, 0:T0W], sb0[:, 0:T0W], Exp,
                                     bias=0.0, scale=SCALE, accum_out=denb)
                nc.vector.reciprocal(rec_pack[:, 0:1], denb)
                nc.vector.reciprocal(rec_pack[0:G, 0:1], den0)
                ab0 = rowsb.tile([P, G + 256], F32, tag="ab")
                nc.gpsimd.tensor_scalar_mul(ab0[:, 0:T0W], pb0[:, 0:T0W],
                                            rec_pack[:, 0:1])
                nc.sync.dma_start(out=attn_ap[h, 0:P, 0:T0W],
                                  in_=ab0[:, 0:T0W])
                # band tiles 1..15
                for t in range(1, NT):
                    wb = 256 if t < NT - 1 else 192
                    c0 = t * P - 64 if t < NT - 1 else S - 192
                    sp = rowp.tile([P, 512], F32, tag="row")
                    msk = maskmid if t < NT - 1 else mask15
                    nc.tensor.matmul(sp[:, 0:G + wb], id_r, msk,
                                     start=True, stop=False)
                    nc.tensor.matmul(sp[:, 0:G], qTh(t * P, (t + 1) * P),
                                     kTh(0, G), start=False, stop=False,
                                     skip_group_check=True)
                    nc.tensor.matmul(sp[:, G:G + wb], qTh(t * P, (t + 1) * P),
                                     kTh(c0, c0 + wb), start=False, stop=True,
                                     skip_group_check=True)
                    pb = rowsb.tile([P, G + 256], F32, tag="pb")
                    den = rowsb.tile([P, 1], F32, tag="den")
                    nc.scalar.activation(pb[:, 0:G + wb], sp[:, 0:G + wb], Exp,
                                         bias=0.0, scale=SCALE, accum_out=den)
                    nc.vector.reciprocal(rec_pack[:, t:t + 1], den)
                    ab = rowsb.tile([P, G + 256], F32, tag="ab")
                    nc.gpsimd.tensor_scalar_mul(ab[:, 0:G + wb], pb[:, 0:G + wb],
                                                rec_pack[:, t:t + 1])
                    nc.sync.dma_start(out=attn_ap[h, t * P:(t + 1) * P, 0:G],
                                      in_=ab[:, 0:G])
                    j0 = t * P - 32 if t < NT - 1 else S - 160
                    wlen = 192 if t < NT - 1 else 160
                    nc.sync.dma_start(
                        out=attn_ap[h, t * P:(t + 1) * P, j0:j0 + wlen],
                        in_=ab[:, G + 32:G + 32 + wlen])

                # ---- recips -> broadcast row; normalize ctxT ----
                rtp = smallp.tile([NT, P], F32, tag="small")
                nc.tensor.transpose(rtp, rec_pack, id32)
                rt_sb = rowsb.tile([NT, P], F32, tag="rt")
                nc.vector.tensor_copy(rt_sb, rtp)
                nc.sync.dma_start(out=recq_ap[h], in_=rt_sb)
                rbc = rowbig.tile([HD, S], F32, tag="rbc")
                rq = recq_ap[h]
                bcast = bass.AP(tensor=rq.tensor, offset=rq.offset,
                                ap=[[0, HD], [1, G * P]])
                nc.sync.dma_start(out=rbc, in_=bcast)
                nc.vector.tensor_mul(ctxT[p0:p0 + HD, mh, :], acc, rbc)
                if "dbg_rt" in T:
                    nc.sync.dma_start(out=T["dbg_rt"].ap()[h], in_=rt_sb)
                    nc.sync.dma_start(out=T["dbg_rbc"].ap()[h], in_=rbc)
                    nc.sync.dma_start(out=T["dbg_rp"].ap()[h], in_=rec_pack)

        if "dbg_qT" in T:
            nc.sync.dma_start(out=T["dbg_qT"].ap(), in_=qT)
            nc.sync.dma_start(out=T["dbg_v"].ap(), in_=v_sb)
            nc.sync.dma_start(out=T["dbg_ctxT"].ap(), in_=ctxT)

        # ================= out projection =================
        tc.no_sync_barrier()
        outT_ap = T["outT_d"].ap()
        with tc.tile_pool(name="opsum", bufs=4, space="PSUM") as opsum, \
             tc.tile_pool(name="osb", bufs=3) as osb:
            for oc in range(8):
                for n in range(4):
                    po = opsum.tile([P, 512], F32, tag="po")
                    for cc in range(2):
                        nc.tensor.matmul(po, wo_sb[:, cc, oc * P:(oc + 1) * P],
                                         ctxT[:, cc, n * 512:(n + 1) * 512],
                                         start=(cc == 0), stop=(cc == 1))
                    o_sb = osb.tile([P, 512], F32, tag="osb")
                    if (oc + n) % 2 == 0:
                        nc.scalar.copy(o_sb, po)
                    else:
                        nc.vector.tensor_copy(o_sb, po)
                    nc.sync.dma_start(
                        out=outT_ap[oc * P:(oc + 1) * P, n * 512:(n + 1) * 512],
                        in_=o_sb)


_CACHE = {}


def _get_module(with_bias):
    if with_bias not in _CACHE:
        _CACHE[with_bias] = _build_module(with_bias)
    return _CACHE[with_bias]


def kernel(query, key, value, Wq, bq, Wk, bk, Wv, bv, Wo, bo):
    query = np.asarray(query, np.float32)
    key = np.asarray(key, np.float32)
    value = np.asarray(value, np.float32)
    Wq, Wk, Wv, Wo = (np.asarray(w, np.float32) for w in (Wq, Wk, Wv, Wo))
    bq, bk, bv, bo = (np.asarray(b, np.float32) for b in (bq, bk, bv, bo))

    with_bias = bool(np.any(bq) or np.any(bk) or np.any(bv))
    nc = _get_module(with_bias)

    mask0b, maskmid, mask15, maskT0, maskT1, maskTi = _build_masks()
    ident = np.eye(P, dtype=np.float32)

    in_maps = []
    for c in range(8):
        b = c // 4
        hs = HC * (c % 4)
        im = {
            "xqT": np.ascontiguousarray(query[b].T),
            "xkT": np.ascontiguousarray(key[b].T),
            "xvT": np.ascontiguousarray(value[b].T),
            "wq": np.ascontiguousarray(Wq[hs:hs + HC].T),
            "wk": np.ascontiguousarray(Wk[hs:hs + HC].T),
            "wv": np.ascontiguousarray(Wv[hs:hs + HC].T),
            "wo": np.ascontiguousarray(Wo[:, hs:hs + HC].T),
            "ident_r": ident, "ident32": ident,
            "mask0b": mask0b, "maskmid": maskmid, "mask15": mask15,
            "maskT0": maskT0, "maskT1": maskT1, "maskTi": maskTi,
        }
        if with_bias:
            im["bq2"] = np.ascontiguousarray(bq[hs:hs + HC].reshape(2, P).T)
            im["bk2"] = np.ascontiguousarray(bk[hs:hs + HC].reshape(2, P).T)
            im["bvb"] = np.tile(bv[hs:hs + HC], (P, 1))
        in_maps.append(im)

    res = run_bass_kernel_spmd(nc, in_maps, core_ids=list(range(8)))

    attn = np.empty((B, H, S, S), np.float32)
    out = np.zeros((B, S, D), np.float32)
    for c in range(8):
        b = c // 4
        g = c % 4
        attn[b, HQ * g:HQ * (g + 1)] = res.results[c]["attn"]
        pex = res.results[c]["pex"]
        attn[b, HQ * g:HQ * (g + 1), 0:G, :] = pex / pex.sum(-1, keepdims=True)
        out[b] += res.results[c]["outT"].T
    out += bo[None, None, :]
    return out, attn


# revision 18
# speedup vs baseline: 2.2137x; 2.2137x over previous
"""Trainium2 Bass kernel for EnhancedAttention (sparse band+global attention).

Problem (hardcoded): B=2, S=2048, D=1024, H=16, HD=64, band |i-j|<=32,
16 global tokens. reference returns (out [B,S,D], attn [B,H,S,S]).

Sharding: 8 cores = 2 batches x 4 head-quads. Core c handles batch c//4,
heads [4*(c%4), 4*(c%4)+4). Per core:
  - project q/k transposed ([256, 2048]) and v natural ([2048, 256]), f32r
  - per (pair) sparse attention:
      row side (attn output): scores + additive -1e9 mask via identity
      matmul into PSUM, exp with fused row-sum (accum_out), reciprocal,
      gpsimd normalize, strided DMA of just the nonzero band/global region
      (output buffers are pre-zeroed by the runtime);
      T side (for ctx^T): scores recomputed transposed in shared 128-col
      chunks, exp, then AV matmuls accumulate ctxT [64, 2048] in PSUM;
      normalization via row-recips transposed + DRAM-broadcast.
  - out-projection partial outT [1024, 2048] = (Wo cols-slice)^T @ ctxT
Host: attn shards concatenated; out = sum of 4 partials per batch + bo.
Masked attn entries are exact zeros (exp underflow), matching reference.
"""
import sys

sys.path.insert(0, "/opt/trn_rl_repo")

import numpy as np

import concourse.bass as bass
import concourse.tile as tile
from concourse import bacc, mybir
from concourse.bass_utils import run_bass_kernel_spmd

B, S, D, H = 2, 2048, 1024, 16
HD = 64          # head dim
HQ = 4           # heads per core
HC = HQ * HD     # 256 head-cols per core
G = 16           # global tokens
P = 128
NT = S // P      # 16 row tiles
KC = D // P      # 8 contraction chunks
F32 = mybir.dt.float32
F32R = mybir.dt.float32r
NEG = np.float32(-1e9)
SCALE = 0.125    # 1/sqrt(64)
T0W = 176        # tile0 band-rows score width (covers band+global of rows 16..127)


def _chunk_rows(m):
    """T-side chunk m: key-cols [128m, 128m+128), query-rows [r0, r1)."""
    return max(0, P * (m - 1)), min(S, P * (m + 2))


def _build_masks():
    """Additive 0/-1e9 masks (f32)."""
    r = np.arange(P)

    def madd(v):
        return np.where(v, np.float32(0.0), NEG)

    # tile0 band-rows piece [128, 176]: i=r, j=c
    c = np.arange(T0W)
    mask0b = madd((np.abs(r[:, None] - c[None, :]) <= 32)
                  | (r[:, None] < G) | (c[None, :] < G))
    # mid tiles 1..14: [16 global][256 window at 128t-64]; j-i = c-r-32-32
    c = np.arange(256)
    band_mid = madd((c[None, :] - r[:, None] >= 32) & (c[None, :] - r[:, None] <= 96))
    maskmid = np.concatenate([np.zeros((P, G), np.float32), band_mid], axis=1)
    # tile 15: [16 global][192 window at 1856]; j=1856+c, i=1920+r
    c = np.arange(192)
    band15 = madd((c[None, :] - r[:, None] >= 32) & (c[None, :] - r[:, None] <= 96))
    mask15 = np.concatenate([np.zeros((P, G), np.float32), band15], axis=1)

    def maskT(m):
        r0, r1 = _chunk_rows(m)
        j = P * m + np.arange(P)
        i = np.arange(r0, r1)
        v = (np.abs(i[None, :] - j[:, None]) <= 32) | (i[None, :] < G)
        v &= j[:, None] >= G
        return madd(v)

    maskT0, maskT1, maskTi = maskT(0), maskT(1), maskT(2)
    for m in range(3, 15):
        assert np.array_equal(maskT(m), maskTi)
    assert np.array_equal(maskT(15), maskTi[:, :256])
    return mask0b, maskmid, mask15, maskT0, maskT1, maskTi


def _build_module(with_bias):
    nc = bacc.Bacc("TRN2", target_bir_lowering=False, debug=False)

    def din(name, shape, dt=F32R):
        return nc.dram_tensor(name, shape, dt, kind="ExternalInput")

    tens = dict(
        xq_d=din("xqT", [D, S]), xk_d=din("xkT", [D, S]), xv_d=din("xvT", [D, S]),
        wq_d=din("wq", [D, HC]), wk_d=din("wk", [D, HC]), wv_d=din("wv", [D, HC]),
        wo_d=din("wo", [HC, D]),
        id_r_d=din("ident_r", [P, P]), id32_d=din("ident32", [P, P], F32),
        m0b_d=din("mask0b", [P, T0W]),
        mm_d=din("maskmid", [P, G + 256]), m15_d=din("mask15", [P, G + 192]),
        mt0_d=din("maskT0", [P, 256]), mt1_d=din("maskT1", [P, 384]),
        mti_d=din("maskTi", [P, 384]),
        attn_d=nc.dram_tensor("attn", [HQ, NT - 1, P, G + 192], F32,
                              kind="ExternalOutput"),
        attn0_d=nc.dram_tensor("attn0", [HQ, P, T0W], F32,
                               kind="ExternalOutput"),
        outT_d=nc.dram_tensor("outT", [D, S], F32, kind="ExternalOutput"),
        pex_d=nc.dram_tensor("pex", [HQ, G, S], F32, kind="ExternalOutput"),
        recq_d=nc.dram_tensor("recq", [HQ, G, P], F32, kind="Internal"),
    )
    import os
    if os.environ.get("KBDEBUG"):
        tens.update(
            dbg_qT=nc.dram_tensor("dbg_qT", [P, 2, S], F32R, kind="ExternalOutput"),
            dbg_v=nc.dram_tensor("dbg_v", [P, NT, HC], F32R, kind="ExternalOutput"),
            dbg_ctxT=nc.dram_tensor("dbg_ctxT", [P, 2, S], F32R, kind="ExternalOutput"),
            dbg_rt=nc.dram_tensor("dbg_rt", [HQ, G, P], F32, kind="ExternalOutput"),
            dbg_rbc=nc.dram_tensor("dbg_rbc", [HQ, HD, S], F32, kind="ExternalOutput"),
            dbg_rp=nc.dram_tensor("dbg_rp", [HQ, P, G], F32, kind="ExternalOutput"),
        )
    if with_bias:
        tens.update(bq_d=din("bq2", [P, 2], F32), bk_d=din("bk2", [P, 2], F32),
                    bv_d=din("bvb", [P, HC], F32))

    with tile.TileContext(nc) as tc:
        _emit(nc, tc, tens, with_bias)
    nc.compile()
    return nc


def _emit(nc, tc, T, with_bias):
    from contextlib import ExitStack
    Exp = mybir.ActivationFunctionType.Exp

    ctx = ExitStack()
    with ctx:
        consts = ctx.enter_context(tc.tile_pool(name="consts", bufs=1))
        qkv = ctx.enter_context(tc.tile_pool(name="qkv", bufs=1))

        # ---- constants ----
        wq_sb = consts.tile([P, KC, HC], F32R)
        wk_sb = consts.tile([P, KC, HC], F32R)
        wv_sb = consts.tile([P, KC, HC], F32R)
        wo_sb = consts.tile([P, 2, D], F32R)
        nc.sync.dma_start(out=wq_sb, in_=T["wq_d"].ap().rearrange(
            "(kc p) n -> p kc n", p=P))
        nc.sync.dma_start(out=wk_sb, in_=T["wk_d"].ap().rearrange(
            "(kc p) n -> p kc n", p=P))
        nc.sync.dma_start(out=wv_sb, in_=T["wv_d"].ap().rearrange(
            "(kc p) n -> p kc n", p=P))
        nc.sync.dma_start(out=wo_sb, in_=T["wo_d"].ap().rearrange(
            "(cc p) n -> p cc n", p=P))

        def load_const(d):
            nm = d.ap().tensor.name
            t = consts.tile(list(d.ap().shape), d.dtype,
                            name=f"c_{nm}", tag=f"c_{nm}")
            nc.sync.dma_start(out=t, in_=d.ap())
            return t

        id_r = load_const(T["id_r_d"])
        id32 = load_const(T["id32_d"])
        mask0b = load_const(T["m0b_d"])
        maskmid = load_const(T["mm_d"])
        mask15 = load_const(T["m15_d"])
        maskT0 = load_const(T["mt0_d"])
        maskT1 = load_const(T["mt1_d"])
        maskTi = load_const(T["mti_d"])
        if with_bias:
            bq_sb = load_const(T["bq_d"])
            bk_sb = load_const(T["bk_d"])
            bv_sb = load_const(T["bv_d"])

        # ---- persistent tensors ----
        qT = qkv.tile([P, 2, S], F32R)
        kT = qkv.tile([P, 2, S], F32R)
        v_sb = qkv.tile([P, NT, HC], F32R)
        ctxT = qkv.tile([P, 2, S], F32R)

        # ================= projections =================
        with tc.tile_pool(name="xpool", bufs=2) as xpool:
            with tc.tile_pool(name="ppsum", bufs=8, space="PSUM") as ppsum:
                for which, x_d, w_sb3, out_t in (
                        ("q", T["xq_d"], wq_sb, qT), ("k", T["xk_d"], wk_sb, kT)):
                    x_view = x_d.ap().rearrange("(kc p) r -> kc p r", p=P)
                    psums = [ppsum.tile([P, 512], F32, tag="pp", name=f"pp_{which}_{i}") for i in range(8)]
                    for kc in range(KC):
                        x_sb = xpool.tile([P, S], F32R, tag="x")
                        nc.sync.dma_start(out=x_sb, in_=x_view[kc])
                        for m in range(2):
                            for n in range(4):
                                nc.tensor.matmul(
                                    psums[m * 4 + n],
                                    w_sb3[:, kc, m * P:(m + 1) * P],
                                    x_sb[:, n * 512:(n + 1) * 512],
                                    start=(kc == 0), stop=(kc == KC - 1))
                    for m in range(2):
                        for n in range(4):
                            dst = out_t[:, m, n * 512:(n + 1) * 512]
                            src = psums[m * 4 + n]
                            if with_bias:
                                bsb = bq_sb if which == "q" else bk_sb
                                nc.vector.tensor_scalar_add(dst, src, bsb[:, m:m + 1])
                            elif n % 2 == 0:
                                nc.scalar.copy(dst, src)
                            else:
                                nc.vector.tensor_copy(dst, src)
            with tc.tile_pool(name="vpsum", bufs=8, space="PSUM") as vpsum:
                x_view = T["xv_d"].ap().rearrange("(kc p) r -> kc p r", p=P)
                zz = xpool.tile([P, 512], F32, tag="zz")
                nc.vector.memset(zz, 0.0)
                vpsums = [vpsum.tile([P, 2, HC], F32, tag="vp", name=f"vp_{i}") for i in range(8)]
                for j in range(8):
                    nc.tensor.matmul(vpsums[j].rearrange("p a b -> p (a b)"),
                                     id32, zz, start=True, stop=False,
                                     skip_group_check=True)
                for kc in range(KC):
                    x_sb = xpool.tile([P, S], F32R, tag="x")
                    nc.sync.dma_start(out=x_sb, in_=x_view[kc])
                    for rt in range(NT):
                        nc.tensor.matmul(
                            vpsums[rt // 2][:, rt % 2, :],
                            x_sb[:, rt * P:(rt + 1) * P],
                            wv_sb[:, kc, :],
                            start=False,
                            stop=(kc == KC - 1 and rt % 2 == 1),
                            skip_group_check=True)
                for rt in range(NT):
                    dst = v_sb[:, rt, :]
                    src_ap = vpsums[rt // 2][:, rt % 2, :]
                    if with_bias:
                        nc.vector.tensor_add(dst, src_ap, bv_sb)
                    elif rt % 2 == 0:
                        nc.scalar.copy(dst, src_ap)
                    else:
                        nc.vector.tensor_copy(dst, src_ap)

        # ================= attention =================
        tc.no_sync_barrier()
        attn_ap = T["attn_d"].ap()
        recq_ap = T["recq_d"].ap()

        with tc.tile_pool(name="stp", bufs=1, space="PSUM") as stp, \
             tc.tile_pool(name="smallp", bufs=1, space="PSUM") as smallp, \
             tc.tile_pool(name="avp", bufs=1, space="PSUM") as avp, \
             tc.tile_pool(name="rowp", bufs=1, space="PSUM") as rowp, \
             tc.tile_pool(name="tp", bufs=3) as tp, \
             tc.tile_pool(name="tg", bufs=1) as tg, \
             tc.tile_pool(name="rowsb", bufs=3) as rowsb, \
             tc.tile_pool(name="rowbig", bufs=1) as rowbig:
            for h in range(HQ):
                p0 = (h % 2) * HD
                mh = h // 2
                hc = h * HD

                def qTh(a, b):
                    return qT[p0:p0 + HD, mh, a:b]

                def kTh(a, b):
                    return kT[p0:p0 + HD, mh, a:b]

                # ---- T-side global block ----
                pTg = tg.tile([G, 4, 512], F32R, tag="pTg")
                for n in range(4):
                    sg = smallp.tile([G, 512], F32, tag="small")
                    nc.tensor.matmul(sg, kTh(0, G), qTh(n * 512, (n + 1) * 512),
                                     start=True, stop=True)
                    nc.scalar.activation(pTg[:, n, :], sg, Exp,
                                         bias=0.0, scale=SCALE)

                # ---- AV init: ctxT_acc [64, 2048] <- v_glob^T @ PTg ----
                acc = avp.tile([HD, S], F32, tag="acc")
                for n in range(4):
                    nc.tensor.matmul(acc[:, n * 512:(n + 1) * 512],
                                     v_sb[0:G, 0, hc:hc + HD], pTg[:, n, :],
                                     start=True, stop=False)

                # ---- T-side chunks (16, mask-first) + AV accumulate ----
                for m in range(NT):
                    r0, r1 = _chunk_rows(m)
                    w = r1 - r0
                    sT = stp.tile([P, 384], F32, tag="sT", name=f"sT_{h}_{m}")
                    pT = tp.tile([P, 384], F32R, tag="pT", name=f"pT_{h}_{m}")
                    mt = (maskT0 if m == 0 else maskT1 if m == 1
                          else maskTi[:, 0:w])
                    nc.tensor.matmul(sT[:, 0:w], id_r, mt,
                                     start=True, stop=False)
                    nc.tensor.matmul(sT[:, 0:w], kTh(m * P, (m + 1) * P),
                                     qTh(r0, r1), start=False, stop=True,
                                     skip_group_check=True)
                    nc.scalar.activation(pT[:, 0:w], sT[:, 0:w], Exp,
                                         bias=0.0, scale=SCALE)
                    nc.tensor.matmul(acc[:, r0:r1], v_sb[:, m, hc:hc + HD],
                                     pT[:, 0:w], start=False, stop=False,
                                     skip_group_check=True)

                # ---- T-side strip (rows 0..16 x cols 256..2048) ----
                ss = smallp.tile([P, 14 * G], F32, tag="small")
                for m in range(2, NT):
                    nc.tensor.matmul(ss[:, (m - 2) * G:(m - 1) * G],
                                     kTh(m * P, (m + 1) * P), qTh(0, G),
                                     start=True, stop=True)
                pTs = tg.tile([P, 14 * G], F32R, tag="pTs")
                nc.scalar.activation(pTs, ss, Exp, bias=0.0, scale=SCALE)
                for m in range(2, NT):
                    nc.tensor.matmul(acc[:, 0:G], v_sb[:, m, hc:hc + HD],
                                     pTs[:, (m - 2) * G:(m - 1) * G],
                                     start=False, stop=(m == NT - 1),
                                     skip_group_check=True)

                # ---- row side ----
                rec_pack = rowsb.tile([P, NT], F32, tag="rpack")
                # tile0 dense rows 0..15: [16, 2048]
                p0d = rowbig.tile([G, 4, 512], F32, tag="p0d")
                den4 = rowsb.tile([G, 4], F32, tag="den4")
                for n in range(4):
                    s0 = smallp.tile([G, 512], F32, tag="small")
                    nc.tensor.matmul(s0, qTh(0, G), kTh(n * 512, (n + 1) * 512),
                                     start=True, stop=True)
                    nc.scalar.activation(p0d[:, n, :], s0, Exp, bias=0.0,
                                         scale=SCALE, accum_out=den4[:, n:n + 1])
                den0 = rowsb.tile([G, 1], F32, tag="dend")
                nc.vector.reduce_sum(den0, den4, axis=mybir.AxisListType.X)
                nc.sync.dma_start(out=T["pex_d"].ap()[h],
                                  in_=p0d.rearrange("g a b -> g (a b)"))
                # tile0 band rows 16..127: [128, 176]
                sb0 = rowp.tile([P, 512], F32, tag="row")
                nc.tensor.matmul(sb0[:, 0:T0W], id_r, mask0b,
                                 start=True, stop=False)
                nc.tensor.matmul(sb0[:, 0:T0W], qTh(0, P), kTh(0, T0W),
                                 start=False, stop=True,
                                 skip_group_check=True)
                pb0 = rowsb.tile([P, G + 256], F32, tag="pb")
                denb = rowsb.tile([P, 1], F32, tag="den")
                nc.scalar.activation(pb0[:, 0:T0W], sb0[:, 0:T0W], Exp,
                                     bias=0.0, scale=SCALE, accum_out=denb)
                nc.vector.reciprocal(rec_pack[:, 0:1], denb)
                nc.vector.reciprocal(rec_pack[0:G, 0:1], den0)
                ab0 = rowsb.tile([P, G + 256], F32, tag="ab")
                nc.gpsimd.tensor_scalar_mul(ab0[:, 0:T0W], pb0[:, 0:T0W],
                                            rec_pack[:, 0:1])
                nc.sync.dma_start(out=T["attn0_d"].ap()[h], in_=ab0[:, 0:T0W])
                # band tiles 1..15
                for t in range(1, NT):
                    wb = 256 if t < NT - 1 else 192
                    c0 = t * P - 64 if t < NT - 1 else S - 192
                    sp = rowp.tile([P, 512], F32, tag="row")
                    msk = maskmid if t < NT - 1 else mask15
                    nc.tensor.matmul(sp[:, 0:G + wb], id_r, msk,
                                     start=True, stop=False)
                    nc.tensor.matmul(sp[:, 0:G], qTh(t * P, (t + 1) * P),
                                     kTh(0, G), start=False, stop=False,
                                     skip_group_check=True)
                    nc.tensor.matmul(sp[:, G:G + wb], qTh(t * P, (t + 1) * P),
                                     kTh(c0, c0 + wb), start=False, stop=True,
                                     skip_group_check=True)
                    pb = rowsb.tile([P, G + 256], F32, tag="pb")
                    den = rowsb.tile([P, 1], F32, tag="den")
                    nc.scalar.activation(pb[:, 0:G + wb], sp[:, 0:G + wb], Exp,
                                         bias=0.0, scale=SCALE, accum_out=den)
                    nc.vector.reciprocal(rec_pack[:, t:t + 1], den)
                    ab = rowsb.tile([P, G + 256], F32, tag="ab")
                    nc.gpsimd.tensor_scalar_mul(ab[:, 0:G + wb], pb[:, 0:G + wb],
                                                rec_pack[:, t:t + 1])
                    wlen = 192 if t < NT - 1 else 160
                    nc.sync.dma_start(out=attn_ap[h, t - 1, :, 0:G],
                                      in_=ab[:, 0:G])
                    nc.sync.dma_start(
                        out=attn_ap[h, t - 1, :, G:G + wlen],
                        in_=ab[:, G + 32:G + 32 + wlen])

                # ---- recips -> broadcast row; normalize ctxT ----
                rtp = smallp.tile([NT, P], F32, tag="small")
                nc.tensor.transpose(rtp, rec_pack, id32)
                rt_sb = rowsb.tile([NT, P], F32, tag="rt")
                nc.vector.tensor_copy(rt_sb, rtp)
                nc.sync.dma_start(out=recq_ap[h], in_=rt_sb)
                rbc = rowbig.tile([HD, S], F32, tag="rbc")
                rq = recq_ap[h]
                bcast = bass.AP(tensor=rq.tensor, offset=rq.offset,
                                ap=[[0, HD], [1, G * P]])
                nc.sync.dma_start(out=rbc, in_=bcast)
                nc.vector.tensor_mul(ctxT[p0:p0 + HD, mh, :], acc, rbc)
                if "dbg_rt" in T:
                    nc.sync.dma_start(out=T["dbg_rt"].ap()[h], in_=rt_sb)
                    nc.sync.dma_start(out=T["dbg_rbc"].ap()[h], in_=rbc)
                    nc.sync.dma_start(out=T["dbg_rp"].ap()[h], in_=rec_pack)

        if "dbg_qT" in T:
            nc.sync.dma_start(out=T["dbg_qT"].ap(), in_=qT)
            nc.sync.dma_start(out=T["dbg_v"].ap(), in_=v_sb)
            nc.sync.dma_start(out=T["dbg_ctxT"].ap(), in_=ctxT)

        # ================= out projection =================
        tc.no_sync_barrier()
        outT_ap = T["outT_d"].ap()
        with tc.tile_pool(name="opsum", bufs=4, space="PSUM") as opsum, \
             tc.tile_pool(name="osb", bufs=3) as osb:
            for oc in range(8):
                for n in range(4):
                    po = opsum.tile([P, 512], F32, tag="po")
                    for cc in range(2):
                        nc.tensor.matmul(po, wo_sb[:, cc, oc * P:(oc + 1) * P],
                                         ctxT[:, cc, n * 512:(n + 1) * 512],
                                         start=(cc == 0), stop=(cc == 1))
                    o_sb = osb.tile([P, 512], F32, tag="osb")
                    if (oc + n) % 2 == 0:
                        nc.scalar.copy(o_sb, po)
                    else:
                        nc.vector.tensor_copy(o_sb, po)
                    nc.sync.dma_start(
                        out=outT_ap[oc * P:(oc + 1) * P, n * 512:(n + 1) * 512],
                        in_=o_sb)


_CACHE = {}
LAST_TIMINGS = {}


def _get_module(with_bias):
    if with_bias not in _CACHE:
        _CACHE[with_bias] = _build_module(with_bias)
    return _CACHE[with_bias]


def kernel(query, key, value, Wq, bq, Wk, bk, Wv, bv, Wo, bo):
    query = np.asarray(query, np.float32)
    key = np.asarray(key, np.float32)
    value = np.asarray(value, np.float32)
    Wq, Wk, Wv, Wo = (np.asarray(w, np.float32) for w in (Wq, Wk, Wv, Wo))
    bq, bk, bv, bo = (np.asarray(b, np.float32) for b in (bq, bk, bv, bo))

    with_bias = bool(np.any(bq) or np.any(bk) or np.any(bv))
    nc = _get_module(with_bias)

    mask0b, maskmid, mask15, maskT0, maskT1, maskTi = _build_masks()
    ident = np.eye(P, dtype=np.float32)

    in_maps = []
    for c in range(8):
        b = c // 4
        hs = HC * (c % 4)
        im = {
            "xqT": np.ascontiguousarray(query[b].T),
            "xkT": np.ascontiguousarray(key[b].T),
            "xvT": np.ascontiguousarray(value[b].T),
            "wq": np.ascontiguousarray(Wq[hs:hs + HC].T),
            "wk": np.ascontiguousarray(Wk[hs:hs + HC].T),
            "wv": np.ascontiguousarray(Wv[hs:hs + HC].T),
            "wo": np.ascontiguousarray(Wo[:, hs:hs + HC].T),
            "ident_r": ident, "ident32": ident,
            "mask0b": mask0b, "maskmid": maskmid, "mask15": mask15,
            "maskT0": maskT0, "maskT1": maskT1, "maskTi": maskTi,
        }
        if with_bias:
            im["bq2"] = np.ascontiguousarray(bq[hs:hs + HC].reshape(2, P).T)
            im["bk2"] = np.ascontiguousarray(bk[hs:hs + HC].reshape(2, P).T)
            im["bvb"] = np.tile(bv[hs:hs + HC], (P, 1))
        in_maps.append(im)

    import time as _time
    _t0 = _time.time()
    res = run_bass_kernel_spmd(nc, in_maps, core_ids=list(range(8)))
    LAST_TIMINGS["run"] = _time.time() - _t0
    _t0 = _time.time()

    attn = np.zeros((B, H, S, S), np.float32)
    out = np.zeros((B, S, D), np.float32)
    for c in range(8):
        b = c // 4
        g = c % 4
        r = res.results[c]
        hs = slice(HQ * g, HQ * (g + 1))
        pex = r["pex"]
        attn[b, hs, 0:G, :] = pex / pex.sum(-1, keepdims=True)
        attn[b, hs, G:P, 0:T0W] = r["attn0"][:, G:P, :]
        band = r["attn"]
        for t in range(1, NT):
            wlen = 192 if t < NT - 1 else 160
            j0 = t * P - 32 if t < NT - 1 else S - 160
            attn[b, hs, t * P:(t + 1) * P, 0:G] = band[:, t - 1, :, 0:G]
            attn[b, hs, t * P:(t + 1) * P, j0:j0 + wlen] = \
                band[:, t - 1, :, G:G + wlen]
        out[b] += r["outT"].T
    out += bo[None, None, :]
    LAST_TIMINGS["assemble"] = _time.time() - _t0
    return out, attn
